# revision 36
# baseline (speedup 1.0000x reference)
"""MultiHeadCrossAttention Trainium2 kernel (8 NeuronCores, SPMD), v2-exact.

Sharding: core c = (batch b=c//4, head-group hg=c%4) -- 4 heads of d=64 each.
Reconstruction of the 310us configuration.
"""

import os
import sys

sys.path.insert(0, "/opt/trn_rl_repo")

import numpy as np
import ml_dtypes

N_HEADS = 16
D = 64
EMB = 1024
CTX = 1024
B = 2
SQ = 2048
SK = 2048
HG = 4
INNER_C = HG * D
EPS = 1e-5
SCALE = 1.0 / 8.0
P = 128

_cached_nc = None


def _build():
    import concourse.bass as bass  # noqa: F401
    import concourse.tile as tile
    from concourse import mybir, bacc
    from contextlib import ExitStack

    f32 = mybir.dt.float32
    bf16 = mybir.dt.bfloat16
    AF = mybir.ActivationFunctionType
    OP = mybir.AluOpType

    nc = bacc.Bacc(None, target_bir_lowering=False, debug=False, num_devices=8)

    embT_d = nc.dram_tensor("embT", [EMB, SQ], bf16, kind="ExternalInput")
    ctxT_d = nc.dram_tensor("ctxT", [CTX, SK], bf16, kind="ExternalInput")
    wqT_d = nc.dram_tensor("wqT", [EMB, INNER_C], bf16, kind="ExternalInput")
    wkT_d = nc.dram_tensor("wkT", [CTX, INNER_C], bf16, kind="ExternalInput")
    wvT_d = nc.dram_tensor("wvT", [CTX, INNER_C], bf16, kind="ExternalInput")
    wu2_d = nc.dram_tensor("wu2", [P, 2, EMB], bf16, kind="ExternalInput")
    red_d = nc.dram_tensor("redblk", [P, 2], bf16, kind="ExternalInput")
    qnw_d = nc.dram_tensor("qnw", [P, 1], f32, kind="ExternalInput")
    qnb_d = nc.dram_tensor("qnb", [P, 1], f32, kind="ExternalInput")
    knw_d = nc.dram_tensor("knw", [P, 1], f32, kind="ExternalInput")
    knb_d = nc.dram_tensor("knb", [P, 1], f32, kind="ExternalInput")
    y_d = nc.dram_tensor("ypart", [SQ, EMB], bf16, kind="ExternalOutput")

    with tile.TileContext(nc) as tc, ExitStack() as top:
        consts = top.enter_context(tc.tile_pool(name="consts", bufs=1))
        red_sb = consts.tile([P, 2], bf16)
        nc.sync.dma_start(red_sb[:], red_d[:])
        qnw_sb = consts.tile([P, 1], f32)
        nc.sync.dma_start(qnw_sb[:], qnw_d[:])
        qnb_sb = consts.tile([P, 1], f32)
        nc.sync.dma_start(qnb_sb[:], qnb_d[:])
        knw_sb = consts.tile([P, 1], f32)
        nc.sync.dma_start(knw_sb[:], knw_d[:])
        knb_sb = consts.tile([P, 1], f32)
        nc.sync.dma_start(knb_sb[:], knb_d[:])
        eps_sb = consts.tile([2, 1], f32)
        nc.vector.memset(eps_sb[:], EPS)
        warm_sb = consts.tile([P, 512], bf16)
        nc.vector.memset(warm_sb[:], 0.0)

        persist = top.enter_context(tc.tile_pool(name="persist", bufs=1))
        qTn_sb = persist.tile([P, 2, SQ], bf16)
        kTn_sb = persist.tile([P, 2, SK], bf16)
        v_sb = persist.tile([P, 16, HG * 65], bf16)
        oT2_sb = persist.tile([P, 2, SQ], bf16)
        wu2_sb = persist.tile([P, 2, EMB], bf16)
        nc.sync.dma_start(wu2_sb[:], wu2_d[:])
        v4 = v_sb.rearrange("p k (g c) -> p k g c", c=65)
        nc.vector.memset(v4[:, :, :, 64:65], 1.0)

        sp_pool = top.enter_context(
            tc.tile_pool(name="sp", bufs=2, space="PSUM"))
        at_pool = top.enter_context(tc.tile_pool(name="at", bufs=36))

        stg = ExitStack()
        with stg:
            xw = stg.enter_context(tc.tile_pool(name="xw", bufs=1))
            ctxT_sb = xw.tile([P, 8, SK], bf16)
            wk_sb = xw.tile([P, 8, INNER_C], bf16, tag="wk")
            nc.sync.dma_start(
                wk_sb[:], wkT_d[:].rearrange("(k p) m -> p k m", p=P))
            for k in range(8):
                nc.sync.dma_start(
                    ctxT_sb[:, k, :],
                    ctxT_d[:].rearrange("(k p) q -> p k q", p=P)[:, k, :])
            embT_sb = xw.tile([P, 8, SQ], bf16, tag="emb")
            wq_sb = xw.tile([P, 8, INNER_C], bf16, tag="wq")
            nc.sync.dma_start(
                wq_sb[:], wqT_d[:].rearrange("(k p) m -> p k m", p=P))
            for k in range(8):
                nc.sync.dma_start(
                    embT_sb[:, k, :],
                    embT_d[:].rearrange("(k p) q -> p k q", p=P)[:, k, :])
            wv_sb = xw.tile([P, 8, INNER_C], bf16, tag="wv")
            nc.sync.dma_start(
                wv_sb[:], wvT_d[:].rearrange("(k p) m -> p k m", p=P))

            pp_pool = stg.enter_context(
                tc.tile_pool(name="pp", bufs=2, space="PSUM"))
            var_pool = stg.enter_context(
                tc.tile_pool(name="var", bufs=2, space="PSUM"))
            lnsb = stg.enter_context(tc.tile_pool(name="lnsb", bufs=3))
            rsb_pool = stg.enter_context(tc.tile_pool(name="rsb", bufs=2))
            drs_pool = stg.enter_context(
                tc.tile_pool(name="drs", bufs=2, space="DRAM"))

            # HAM warm-up: dependency-free matmuls keep the PE clock at
            # 2.4GHz through the input-DMA wait so projections start warm
            for j in range(72):
                jp = pp_pool.tile([P, 512], f32, tag="pp", name=f"jw{j}")
                nc.tensor.matmul(jp[0:2, :], red_sb[:], warm_sb[:],
                                 start=True, stop=True)

            def proj_ln(xT, w_sb_, out_sb, mc, w_ap, b_ap):
                for c in range(4):
                    pp = pp_pool.tile([P, 512], f32, tag="pp")
                    for k in range(8):
                        nc.tensor.matmul(
                            pp[:],
                            w_sb_[:, k, 128 * mc:128 * mc + 128],
                            xT[:, k, 512 * c:512 * c + 512],
                            start=(k == 0), stop=(k == 7))
                    xo = out_sb[:, mc, 512 * c:512 * c + 512]
                    nc.vector.tensor_copy(xo, pp[:])
                    sq = lnsb.tile([P, 512], bf16, tag="sq")
                    nc.vector.tensor_mul(sq[:], xo, xo)
                    vr = var_pool.tile([2, 512], f32, tag="vr")
                    nc.tensor.matmul(vr[:], red_sb[:], sq[:],
                                     start=True, stop=True)
                    lnt = lnsb.tile([2, 512], f32, tag="lnt")
                    nc.scalar.activation(lnt[:], vr[:], AF.Ln, bias=eps_sb[:])
                    rs = lnsb.tile([2, 512], f32, tag="rs")
                    nc.scalar.activation(rs[:], lnt[:], AF.Exp, scale=-0.5)
                    rsd = drs_pool.tile([2, 512], f32)
                    nc.sync.dma_start(rsd[:], rs[:])
                    rsb = rsb_pool.tile([P, 512], f32)
                    nc.sync.dma_start(
                        rsb[0:64, :], rsd[0:1, :].to_broadcast((64, 512)))
                    nc.sync.dma_start(
                        rsb[64:128, :], rsd[1:2, :].to_broadcast((64, 512)))
                    nc.vector.scalar_tensor_tensor(
                        xo, xo, w_ap, rsb[:], op0=OP.mult, op1=OP.mult)
                    nc.vector.tensor_scalar_add(xo, xo, b_ap)

            proj_ln(ctxT_sb, wk_sb, kTn_sb, 0, knw_sb[:], knb_sb[:])
            proj_ln(embT_sb, wq_sb, qTn_sb, 0, qnw_sb[:], qnb_sb[:])
            proj_ln(ctxT_sb, wk_sb, kTn_sb, 1, knw_sb[:], knb_sb[:])
            proj_ln(embT_sb, wq_sb, qTn_sb, 1, qnw_sb[:], qnb_sb[:])

            def scores_unit(qh, hp):
                ats = {}
                for kt in range(16):
                    for h2 in range(2):
                        po = 64 * h2
                        sp = sp_pool.tile([P, 1024], f32, tag="sp")
                        for qn in range(2):
                            nc.tensor.matmul(
                                sp[:, 512 * qn:512 * qn + 512],
                                kTn_sb[po:po + 64, hp,
                                       128 * kt:128 * kt + 128],
                                qTn_sb[po:po + 64, hp,
                                       1024 * qh + 512 * qn:
                                       1024 * qh + 512 * qn + 512],
                                start=True, stop=True,
                                tile_position=(po, 0))
                        at = at_pool.tile([P, 1024], bf16)
                        nc.scalar.activation(at[:], sp[:], AF.Exp,
                                             scale=SCALE)
                        ats[(h2, kt)] = at
                return ats

            ats00 = scores_unit(0, 0)

            for sk in range(16):
                vp = pp_pool.tile([P, 256], f32, tag="pp")
                for k in range(8):
                    nc.tensor.matmul(
                        vp[:],
                        ctxT_sb[:, k, 128 * sk:128 * sk + 128],
                        wv_sb[:, k, :],
                        start=(k == 0), stop=(k == 7))
                nc.vector.tensor_copy(
                    v4[:, sk, :, 0:64],
                    vp[:].rearrange("p (g c) -> p g c", c=64))

        with ExitStack() as sb2:
            ot_pool = sb2.enter_context(
                tc.tile_pool(name="ot", bufs=4, space="PSUM"))
            den_pool = sb2.enter_context(tc.tile_pool(name="den", bufs=2))
            dend_pool = sb2.enter_context(
                tc.tile_pool(name="dend", bufs=2, space="DRAM"))
            obc_pool = sb2.enter_context(tc.tile_pool(name="obc", bufs=4))
            tmp_pool = sb2.enter_context(tc.tile_pool(name="tmp", bufs=2))
            y_pool = sb2.enter_context(tc.tile_pool(name="yo", bufs=3))

            last_tmp = [None]

            def av_unit(qh, hp, ats):
                denall = den_pool.tile([65, 2048], f32, tag="denall")
                ots = {}
                for h2 in range(2):
                    for qc2 in range(2):
                        j = 2 * h2 + qc2
                        ot = ot_pool.tile([65, 512], f32, tag="ot",
                                          name=f"ot{j}")
                        h = 2 * hp + h2
                        for kt in range(16):
                            nc.tensor.matmul(
                                ot[:],
                                v_sb[:, kt, 65 * h:65 * h + 65],
                                ats[(h2, kt)][:, 512 * qc2:512 * qc2 + 512],
                                start=(kt == 0), stop=(kt == 15))
                        nc.vector.tensor_copy(
                            denall[64:65, 512 * j:512 * j + 512],
                            ot[64:65, :])
                        ots[j] = ot
                dend = dend_pool.tile([1, 2048], f32, tag="dend")
                nc.sync.dma_start(dend[:], denall[64:65, :])
                den4 = den_pool.tile([4, 512], f32, tag="den4")
                nc.sync.dma_start(
                    den4[:],
                    dend[0:1, :].rearrange("p (i c) -> (p i) c", c=512))
                den4r = den_pool.tile([4, 512], f32, tag="den4r")
                nc.vector.reciprocal_approx_fast(den4r[:], den4[:])
                dend2 = dend_pool.tile([4, 512], f32, tag="dend2")
                nc.sync.dma_start(dend2[:], den4r[:])
                for h2 in range(2):
                    for qc2 in range(2):
                        j = 2 * h2 + qc2
                        qc = 2 * qh + qc2
                        obc = obc_pool.tile([64, 512], f32)
                        nc.sync.dma_start(
                            obc[:], dend2[j:j + 1, :].to_broadcast((64, 512)))
                        if h2 == 0:
                            nc.vector.tensor_mul(
                                oT2_sb[0:64, hp, 512 * qc:512 * qc + 512],
                                ots[j][0:64, :], obc[:])
                        else:
                            tmp = tmp_pool.tile([64, 512], bf16)
                            nc.vector.tensor_mul(tmp[:], ots[j][0:64, :],
                                                 obc[:])
                            nc.sync.dma_start(
                                oT2_sb[64:128, hp, 512 * qc:512 * qc + 512],
                                tmp[:])
                            last_tmp[0] = tmp

            def out_proj(qh):
                for qt in range(8):
                    q0 = 1024 * qh + 128 * qt
                    yp = sp_pool.tile([P, 1024], f32, tag="sp")
                    for hp in range(2):
                        for e2 in range(2):
                            nc.tensor.matmul(
                                yp[:, 512 * e2:512 * e2 + 512],
                                oT2_sb[:, hp, q0:q0 + 128],
                                wu2_sb[:, hp, 512 * e2:512 * e2 + 512],
                                start=(hp == 0), stop=(hp == 1))
                    ysb = y_pool.tile([P, 1024], bf16)
                    nc.vector.tensor_copy(ysb[:], yp[:])
                    nc.sync.dma_start(y_d[q0:q0 + 128, :], ysb[:])

            av_unit(0, 0, ats00)
            ats01 = scores_unit(0, 1)
            av_unit(0, 1, ats01)
            ats10 = scores_unit(1, 0)
            av_unit(1, 0, ats10)
            out_proj(0)
            ats11 = scores_unit(1, 1)
            av_unit(1, 1, ats11)
            # tail warm-up: chained on the last unit's norm output so these
            # fire during the den-chain gap, keeping PE warm for out_proj(1)
            for j in range(20):
                jp = sp_pool.tile([P, 1024], f32, tag="sp", name=f"jt{j}")
                nc.tensor.matmul(jp[0:2, 0:512], red_sb[0:64, :],
                                 last_tmp[0][:], start=True, stop=True)
            out_proj(1)

    nc.compile()
    return nc


def _host_inputs(emb, context, Wq, Wk, Wv, Wu, qn_w, qn_b, kn_w, kn_b):
    bf16 = ml_dtypes.bfloat16
    redblk = np.zeros((P, 2), np.float32)
    redblk[0:64, 0] = 1.0 / 64.0
    redblk[64:128, 1] = 1.0 / 64.0
    redblk = redblk.astype(bf16)

    def center(Wrows):
        Wh = Wrows.reshape(HG, D, Wrows.shape[1])
        return (Wh - Wh.mean(axis=1, keepdims=True)).reshape(Wrows.shape)

    tile2 = lambda w: np.ascontiguousarray(
        np.tile(np.asarray(w, np.float32), 2)[:, None])

    in_maps = []
    for c in range(8):
        b, hg = divmod(c, 4)
        rows = slice(INNER_C * hg, INNER_C * (hg + 1))
        wu2 = np.stack([
            np.ascontiguousarray(Wu[:, INNER_C * hg + 128 * hp:
                                    INNER_C * hg + 128 * hp + 128].T)
            for hp in range(2)], axis=1)
        in_maps.append({
            "embT": np.ascontiguousarray(emb[b].T).astype(bf16),
            "ctxT": np.ascontiguousarray(context[b].T).astype(bf16),
            "wqT": np.ascontiguousarray(center(Wq[rows]).T).astype(bf16),
            "wkT": np.ascontiguousarray(center(Wk[rows]).T).astype(bf16),
            "wvT": np.ascontiguousarray(Wv[rows].T).astype(bf16),
            "wu2": np.ascontiguousarray(wu2).astype(bf16),
            "redblk": redblk,
            "qnw": tile2(qn_w),
            "qnb": tile2(qn_b),
            "knw": tile2(kn_w),
            "knb": tile2(kn_b),
        })
    return in_maps


def kernel(emb, context, Wq, Wk, Wv, Wu, bu, qn_w, qn_b, kn_w, kn_b):
    from concourse.bass_utils import run_bass_kernel_spmd

    global _cached_nc
    if _cached_nc is None:
        _cached_nc = _build()
    nc = _cached_nc

    in_maps = _host_inputs(np.asarray(emb, np.float32),
                           np.asarray(context, np.float32),
                           np.asarray(Wq), np.asarray(Wk), np.asarray(Wv),
                           np.asarray(Wu), np.asarray(qn_w), np.asarray(qn_b),
                           np.asarray(kn_w), np.asarray(kn_b))

    trace = bool(os.environ.get("KERNEL_TRACE"))
    res = run_bass_kernel_spmd(nc, in_maps, core_ids=list(range(8)),
                               trace=trace)
    if trace:
        print(f"HW exec time: {res.exec_time_ns} ns")

    out = np.zeros((B, SQ, EMB), np.float32)
    for c in range(8):
        out[c // 4] += np.asarray(res.results[c]["ypart"], np.float32)
    out += np.asarray(bu, np.float32)[None, None, :]
    return out


if __name__ == "__main__":
    pass


# revision 37
# speedup vs baseline: 1.0304x; 1.0304x over previous
"""MultiHeadCrossAttention Trainium2 kernel (8 NeuronCores, SPMD), v2-exact.

Sharding: core c = (batch b=c//4, head-group hg=c%4) -- 4 heads of d=64 each.
Reconstruction of the 310us configuration.
"""

import os
import sys

sys.path.insert(0, "/opt/trn_rl_repo")

import numpy as np
import ml_dtypes

N_HEADS = 16
D = 64
EMB = 1024
CTX = 1024
B = 2
SQ = 2048
SK = 2048
HG = 4
INNER_C = HG * D
EPS = 1e-5
SCALE = 1.0 / 8.0
P = 128

_cached_nc = None


def _build():
    import concourse.bass as bass  # noqa: F401
    import concourse.tile as tile
    from concourse import mybir, bacc
    from contextlib import ExitStack

    f32 = mybir.dt.float32
    bf16 = mybir.dt.bfloat16
    AF = mybir.ActivationFunctionType
    OP = mybir.AluOpType

    nc = bacc.Bacc(None, target_bir_lowering=False, debug=False, num_devices=8)

    embT_d = nc.dram_tensor("embT", [EMB, SQ], bf16, kind="ExternalInput")
    ctxT_d = nc.dram_tensor("ctxT", [CTX, SK], bf16, kind="ExternalInput")
    wqT_d = nc.dram_tensor("wqT", [EMB, INNER_C], bf16, kind="ExternalInput")
    wkT_d = nc.dram_tensor("wkT", [CTX, INNER_C], bf16, kind="ExternalInput")
    wvT_d = nc.dram_tensor("wvT", [CTX, INNER_C], bf16, kind="ExternalInput")
    wu2_d = nc.dram_tensor("wu2", [P, 2, EMB], bf16, kind="ExternalInput")
    red_d = nc.dram_tensor("redblk", [P, 2], bf16, kind="ExternalInput")
    qnw_d = nc.dram_tensor("qnw", [P, 1], f32, kind="ExternalInput")
    qnb_d = nc.dram_tensor("qnb", [P, 1], f32, kind="ExternalInput")
    knw_d = nc.dram_tensor("knw", [P, 1], f32, kind="ExternalInput")
    knb_d = nc.dram_tensor("knb", [P, 1], f32, kind="ExternalInput")
    y_d = nc.dram_tensor("ypart", [SQ, EMB], bf16, kind="ExternalOutput")

    with tile.TileContext(nc) as tc, ExitStack() as top:
        consts = top.enter_context(tc.tile_pool(name="consts", bufs=1))
        red_sb = consts.tile([P, 2], bf16)
        nc.sync.dma_start(red_sb[:], red_d[:])
        qnw_sb = consts.tile([P, 1], f32)
        nc.sync.dma_start(qnw_sb[:], qnw_d[:])
        qnb_sb = consts.tile([P, 1], f32)
        nc.sync.dma_start(qnb_sb[:], qnb_d[:])
        knw_sb = consts.tile([P, 1], f32)
        nc.sync.dma_start(knw_sb[:], knw_d[:])
        knb_sb = consts.tile([P, 1], f32)
        nc.sync.dma_start(knb_sb[:], knb_d[:])
        eps_sb = consts.tile([2, 1], f32)
        nc.vector.memset(eps_sb[:], EPS)

        persist = top.enter_context(tc.tile_pool(name="persist", bufs=1))
        qTn_sb = persist.tile([P, 2, SQ], bf16)
        kTn_sb = persist.tile([P, 2, SK], bf16)
        v_sb = persist.tile([P, 16, HG * 65], bf16)
        oT2_sb = persist.tile([P, 2, SQ], bf16)
        wu2_sb = persist.tile([P, 2, EMB], bf16)
        nc.sync.dma_start(wu2_sb[:], wu2_d[:])
        v4 = v_sb.rearrange("p k (g c) -> p k g c", c=65)
        nc.vector.memset(v4[:, :, :, 64:65], 1.0)

        sp_pool = top.enter_context(
            tc.tile_pool(name="sp", bufs=2, space="PSUM"))
        at_pool = top.enter_context(tc.tile_pool(name="at", bufs=36))

        stg = ExitStack()
        with stg:
            xw = stg.enter_context(tc.tile_pool(name="xw", bufs=1))
            ctxT_sb = xw.tile([P, 8, SK], bf16)
            wk_sb = xw.tile([P, 8, INNER_C], bf16, tag="wk")
            nc.sync.dma_start(
                wk_sb[:], wkT_d[:].rearrange("(k p) m -> p k m", p=P))
            for k in range(8):
                nc.sync.dma_start(
                    ctxT_sb[:, k, :],
                    ctxT_d[:].rearrange("(k p) q -> p k q", p=P)[:, k, :])
            embT_sb = xw.tile([P, 8, SQ], bf16, tag="emb")
            wq_sb = xw.tile([P, 8, INNER_C], bf16, tag="wq")
            nc.sync.dma_start(
                wq_sb[:], wqT_d[:].rearrange("(k p) m -> p k m", p=P))
            for k in range(8):
                nc.sync.dma_start(
                    embT_sb[:, k, :],
                    embT_d[:].rearrange("(k p) q -> p k q", p=P)[:, k, :])
            wv_sb = xw.tile([P, 8, INNER_C], bf16, tag="wv")
            nc.sync.dma_start(
                wv_sb[:], wvT_d[:].rearrange("(k p) m -> p k m", p=P))

            pp_pool = stg.enter_context(
                tc.tile_pool(name="pp", bufs=2, space="PSUM"))
            var_pool = stg.enter_context(
                tc.tile_pool(name="var", bufs=2, space="PSUM"))
            lnsb = stg.enter_context(tc.tile_pool(name="lnsb", bufs=3))
            rsb_pool = stg.enter_context(tc.tile_pool(name="rsb", bufs=2))
            drs_pool = stg.enter_context(
                tc.tile_pool(name="drs", bufs=2, space="DRAM"))

            def proj_ln(xT, w_sb_, out_sb, mc, w_ap, b_ap):
                for c in range(4):
                    pp = pp_pool.tile([P, 512], f32, tag="pp")
                    for k in range(8):
                        nc.tensor.matmul(
                            pp[:],
                            w_sb_[:, k, 128 * mc:128 * mc + 128],
                            xT[:, k, 512 * c:512 * c + 512],
                            start=(k == 0), stop=(k == 7))
                    xo = out_sb[:, mc, 512 * c:512 * c + 512]
                    nc.vector.tensor_copy(xo, pp[:])
                    sq = lnsb.tile([P, 512], bf16, tag="sq")
                    nc.vector.tensor_mul(sq[:], xo, xo)
                    vr = var_pool.tile([2, 512], f32, tag="vr")
                    nc.tensor.matmul(vr[:], red_sb[:], sq[:],
                                     start=True, stop=True)
                    lnt = lnsb.tile([2, 512], f32, tag="lnt")
                    nc.scalar.activation(lnt[:], vr[:], AF.Ln, bias=eps_sb[:])
                    rs = lnsb.tile([2, 512], f32, tag="rs")
                    nc.scalar.activation(rs[:], lnt[:], AF.Exp, scale=-0.5)
                    rsd = drs_pool.tile([2, 512], f32)
                    nc.sync.dma_start(rsd[:], rs[:])
                    rsb = rsb_pool.tile([P, 512], f32)
                    nc.sync.dma_start(
                        rsb[0:64, :], rsd[0:1, :].to_broadcast((64, 512)))
                    nc.sync.dma_start(
                        rsb[64:128, :], rsd[1:2, :].to_broadcast((64, 512)))
                    nc.vector.scalar_tensor_tensor(
                        xo, xo, w_ap, rsb[:], op0=OP.mult, op1=OP.mult)
                    nc.vector.tensor_scalar_add(xo, xo, b_ap)

            proj_ln(ctxT_sb, wk_sb, kTn_sb, 0, knw_sb[:], knb_sb[:])
            proj_ln(embT_sb, wq_sb, qTn_sb, 0, qnw_sb[:], qnb_sb[:])
            proj_ln(ctxT_sb, wk_sb, kTn_sb, 1, knw_sb[:], knb_sb[:])
            proj_ln(embT_sb, wq_sb, qTn_sb, 1, qnw_sb[:], qnb_sb[:])

            def scores_unit(qh, hp):
                ats = {}
                for kt in range(16):
                    for h2 in range(2):
                        po = 64 * h2
                        sp = sp_pool.tile([P, 1024], f32, tag="sp")
                        for qn in range(2):
                            nc.tensor.matmul(
                                sp[:, 512 * qn:512 * qn + 512],
                                kTn_sb[po:po + 64, hp,
                                       128 * kt:128 * kt + 128],
                                qTn_sb[po:po + 64, hp,
                                       1024 * qh + 512 * qn:
                                       1024 * qh + 512 * qn + 512],
                                start=True, stop=True,
                                tile_position=(po, 0))
                        at = at_pool.tile([P, 1024], bf16)
                        nc.scalar.activation(at[:], sp[:], AF.Exp,
                                             scale=SCALE)
                        ats[(h2, kt)] = at
                return ats

            ats00 = scores_unit(0, 0)

            for sk in range(16):
                vp = pp_pool.tile([P, 256], f32, tag="pp")
                for k in range(8):
                    nc.tensor.matmul(
                        vp[:],
                        ctxT_sb[:, k, 128 * sk:128 * sk + 128],
                        wv_sb[:, k, :],
                        start=(k == 0), stop=(k == 7))
                nc.vector.tensor_copy(
                    v4[:, sk, :, 0:64],
                    vp[:].rearrange("p (g c) -> p g c", c=64))

        with ExitStack() as sb2:
            ot_pool = sb2.enter_context(
                tc.tile_pool(name="ot", bufs=4, space="PSUM"))
            den_pool = sb2.enter_context(tc.tile_pool(name="den", bufs=2))
            dend_pool = sb2.enter_context(
                tc.tile_pool(name="dend", bufs=2, space="DRAM"))
            obc_pool = sb2.enter_context(tc.tile_pool(name="obc", bufs=4))
            tmp_pool = sb2.enter_context(tc.tile_pool(name="tmp", bufs=2))
            y_pool = sb2.enter_context(tc.tile_pool(name="yo", bufs=3))

            def av_unit(qh, hp, ats):
                denall = den_pool.tile([65, 2048], f32, tag="denall")
                ots = {}
                for h2 in range(2):
                    for qc2 in range(2):
                        j = 2 * h2 + qc2
                        ot = ot_pool.tile([65, 512], f32, tag="ot",
                                          name=f"ot{j}")
                        h = 2 * hp + h2
                        for kt in range(16):
                            nc.tensor.matmul(
                                ot[:],
                                v_sb[:, kt, 65 * h:65 * h + 65],
                                ats[(h2, kt)][:, 512 * qc2:512 * qc2 + 512],
                                start=(kt == 0), stop=(kt == 15))
                        nc.vector.tensor_copy(
                            denall[64:65, 512 * j:512 * j + 512],
                            ot[64:65, :])
                        ots[j] = ot
                dend = dend_pool.tile([1, 2048], f32, tag="dend")
                nc.sync.dma_start(dend[:], denall[64:65, :])
                den4 = den_pool.tile([4, 512], f32, tag="den4")
                nc.sync.dma_start(
                    den4[:],
                    dend[0:1, :].rearrange("p (i c) -> (p i) c", c=512))
                den4r = den_pool.tile([4, 512], f32, tag="den4r")
                nc.vector.reciprocal_approx_fast(den4r[:], den4[:])
                dend2 = dend_pool.tile([4, 512], f32, tag="dend2")
                nc.sync.dma_start(dend2[:], den4r[:])
                for h2 in range(2):
                    for qc2 in range(2):
                        j = 2 * h2 + qc2
                        qc = 2 * qh + qc2
                        obc = obc_pool.tile([64, 512], f32)
                        nc.sync.dma_start(
                            obc[:], dend2[j:j + 1, :].to_broadcast((64, 512)))
                        if h2 == 0:
                            nc.vector.tensor_mul(
                                oT2_sb[0:64, hp, 512 * qc:512 * qc + 512],
                                ots[j][0:64, :], obc[:])
                        else:
                            tmp = tmp_pool.tile([64, 512], bf16)
                            nc.vector.tensor_mul(tmp[:], ots[j][0:64, :],
                                                 obc[:])
                            nc.sync.dma_start(
                                oT2_sb[64:128, hp, 512 * qc:512 * qc + 512],
                                tmp[:])

            def out_proj(qh):
                for qt in range(8):
                    q0 = 1024 * qh + 128 * qt
                    yp = sp_pool.tile([P, 1024], f32, tag="sp")
                    for hp in range(2):
                        for e2 in range(2):
                            nc.tensor.matmul(
                                yp[:, 512 * e2:512 * e2 + 512],
                                oT2_sb[:, hp, q0:q0 + 128],
                                wu2_sb[:, hp, 512 * e2:512 * e2 + 512],
                                start=(hp == 0), stop=(hp == 1))
                    ysb = y_pool.tile([P, 1024], bf16)
                    nc.vector.tensor_copy(ysb[:], yp[:])
                    nc.sync.dma_start(y_d[q0:q0 + 128, :], ysb[:])

            av_unit(0, 0, ats00)
            ats01 = scores_unit(0, 1)
            av_unit(0, 1, ats01)
            ats10 = scores_unit(1, 0)
            av_unit(1, 0, ats10)
            out_proj(0)
            ats11 = scores_unit(1, 1)
            av_unit(1, 1, ats11)
            out_proj(1)

    nc.compile()
    return nc


def _host_inputs(emb, context, Wq, Wk, Wv, Wu, qn_w, qn_b, kn_w, kn_b):
    bf16 = ml_dtypes.bfloat16
    redblk = np.zeros((P, 2), np.float32)
    redblk[0:64, 0] = 1.0 / 64.0
    redblk[64:128, 1] = 1.0 / 64.0
    redblk = redblk.astype(bf16)

    def center(Wrows):
        Wh = Wrows.reshape(HG, D, Wrows.shape[1])
        return (Wh - Wh.mean(axis=1, keepdims=True)).reshape(Wrows.shape)

    tile2 = lambda w: np.ascontiguousarray(
        np.tile(np.asarray(w, np.float32), 2)[:, None])

    in_maps = []
    for c in range(8):
        b, hg = divmod(c, 4)
        rows = slice(INNER_C * hg, INNER_C * (hg + 1))
        wu2 = np.stack([
            np.ascontiguousarray(Wu[:, INNER_C * hg + 128 * hp:
                                    INNER_C * hg + 128 * hp + 128].T)
            for hp in range(2)], axis=1)
        in_maps.append({
            "embT": np.ascontiguousarray(emb[b].T).astype(bf16),
            "ctxT": np.ascontiguousarray(context[b].T).astype(bf16),
            "wqT": np.ascontiguousarray(center(Wq[rows]).T).astype(bf16),
            "wkT": np.ascontiguousarray(center(Wk[rows]).T).astype(bf16),
            "wvT": np.ascontiguousarray(Wv[rows].T).astype(bf16),
            "wu2": np.ascontiguousarray(wu2).astype(bf16),
            "redblk": redblk,
            "qnw": tile2(qn_w),
            "qnb": tile2(qn_b),
            "knw": tile2(kn_w),
            "knb": tile2(kn_b),
        })
    return in_maps


def kernel(emb, context, Wq, Wk, Wv, Wu, bu, qn_w, qn_b, kn_w, kn_b):
    from concourse.bass_utils import run_bass_kernel_spmd

    global _cached_nc
    if _cached_nc is None:
        _cached_nc = _build()
    nc = _cached_nc

    in_maps = _host_inputs(np.asarray(emb, np.float32),
                           np.asarray(context, np.float32),
                           np.asarray(Wq), np.asarray(Wk), np.asarray(Wv),
                           np.asarray(Wu), np.asarray(qn_w), np.asarray(qn_b),
                           np.asarray(kn_w), np.asarray(kn_b))

    trace = bool(os.environ.get("KERNEL_TRACE"))
    res = run_bass_kernel_spmd(nc, in_maps, core_ids=list(range(8)),
                               trace=trace)
    if trace:
        print(f"HW exec time: {res.exec_time_ns} ns")

    out = np.zeros((B, SQ, EMB), np.float32)
    for c in range(8):
        out[c // 4] += np.asarray(res.results[c]["ypart"], np.float32)
    out += np.asarray(bu, np.float32)[None, None, :]
    return out


if __name__ == "__main__":
    pass


# revision 38
# speedup vs baseline: 1.1194x; 1.0864x over previous
"""MultiHeadCrossAttention Trainium2 kernel (8 NeuronCores, SPMD), v2-exact.

Sharding: core c = (batch b=c//4, head-group hg=c%4) -- 4 heads of d=64 each.
Reconstruction of the 310us configuration.
"""

import os
import sys

sys.path.insert(0, "/opt/trn_rl_repo")

import numpy as np
import ml_dtypes

N_HEADS = 16
D = 64
EMB = 1024
CTX = 1024
B = 2
SQ = 2048
SK = 2048
HG = 4
INNER_C = HG * D
EPS = 1e-5
SCALE = 1.0 / 8.0
P = 128

_cached_nc = None


def _build():
    import concourse.bass as bass  # noqa: F401
    import concourse.tile as tile
    from concourse import mybir, bacc
    from contextlib import ExitStack

    f32 = mybir.dt.float32
    bf16 = mybir.dt.bfloat16
    AF = mybir.ActivationFunctionType
    OP = mybir.AluOpType

    nc = bacc.Bacc(None, target_bir_lowering=False, debug=False, num_devices=8)

    embT_d = nc.dram_tensor("embT", [EMB, SQ], bf16, kind="ExternalInput")
    ctxT_d = nc.dram_tensor("ctxT", [CTX, SK], bf16, kind="ExternalInput")
    wqT_d = nc.dram_tensor("wqT", [EMB, INNER_C], bf16, kind="ExternalInput")
    wkT_d = nc.dram_tensor("wkT", [CTX, INNER_C], bf16, kind="ExternalInput")
    wvT_d = nc.dram_tensor("wvT", [CTX, INNER_C], bf16, kind="ExternalInput")
    wu2_d = nc.dram_tensor("wu2", [P, 2, EMB], bf16, kind="ExternalInput")
    red_d = nc.dram_tensor("redblk", [P, 2], bf16, kind="ExternalInput")
    qnw_d = nc.dram_tensor("qnw", [P, 1], f32, kind="ExternalInput")
    qnb_d = nc.dram_tensor("qnb", [P, 1], f32, kind="ExternalInput")
    knw_d = nc.dram_tensor("knw", [P, 1], f32, kind="ExternalInput")
    knb_d = nc.dram_tensor("knb", [P, 1], f32, kind="ExternalInput")
    y_d = nc.dram_tensor("ypart", [SQ, EMB], bf16, kind="ExternalOutput")

    with tile.TileContext(nc) as tc, ExitStack() as top:
        consts = top.enter_context(tc.tile_pool(name="consts", bufs=1))
        red_sb = consts.tile([P, 2], bf16)
        nc.sync.dma_start(red_sb[:], red_d[:])
        qnw_sb = consts.tile([P, 1], f32)
        nc.sync.dma_start(qnw_sb[:], qnw_d[:])
        qnb_sb = consts.tile([P, 1], f32)
        nc.sync.dma_start(qnb_sb[:], qnb_d[:])
        knw_sb = consts.tile([P, 1], f32)
        nc.sync.dma_start(knw_sb[:], knw_d[:])
        knb_sb = consts.tile([P, 1], f32)
        nc.sync.dma_start(knb_sb[:], knb_d[:])
        eps_sb = consts.tile([2, 1], f32)
        nc.vector.memset(eps_sb[:], EPS)

        persist = top.enter_context(tc.tile_pool(name="persist", bufs=1))
        qTn_sb = persist.tile([P, 2, SQ], bf16)
        kTn_sb = persist.tile([P, 2, SK], bf16)
        v_sb = persist.tile([P, 16, HG * 65], bf16)
        oT2_sb = persist.tile([P, 2, SQ], bf16)
        wu2_sb = persist.tile([P, 2, EMB], bf16)
        nc.sync.dma_start(wu2_sb[:], wu2_d[:])
        v4 = v_sb.rearrange("p k (g c) -> p k g c", c=65)
        nc.vector.memset(v4[:, :, :, 64:65], 1.0)

        sp_pool = top.enter_context(
            tc.tile_pool(name="sp", bufs=2, space="PSUM"))
        at_pool = top.enter_context(tc.tile_pool(name="at", bufs=36))

        stg = ExitStack()
        with stg:
            xw = stg.enter_context(tc.tile_pool(name="xw", bufs=1))
            ctxT_sb = xw.tile([P, 8, SK], bf16)
            wk_sb = xw.tile([P, 8, INNER_C], bf16, tag="wk")
            nc.sync.dma_start(
                wk_sb[:], wkT_d[:].rearrange("(k p) m -> p k m", p=P))
            for k in range(8):
                nc.sync.dma_start(
                    ctxT_sb[:, k, :],
                    ctxT_d[:].rearrange("(k p) q -> p k q", p=P)[:, k, :])
            embT_sb = xw.tile([P, 8, SQ], bf16, tag="emb")
            wq_sb = xw.tile([P, 8, INNER_C], bf16, tag="wq")
            nc.sync.dma_start(
                wq_sb[:], wqT_d[:].rearrange("(k p) m -> p k m", p=P))
            for k in range(8):
                nc.sync.dma_start(
                    embT_sb[:, k, :],
                    embT_d[:].rearrange("(k p) q -> p k q", p=P)[:, k, :])
            wv_sb = xw.tile([P, 8, INNER_C], bf16, tag="wv")
            nc.sync.dma_start(
                wv_sb[:], wvT_d[:].rearrange("(k p) m -> p k m", p=P))

            pp_pool = stg.enter_context(
                tc.tile_pool(name="pp", bufs=2, space="PSUM"))
            var_pool = stg.enter_context(
                tc.tile_pool(name="var", bufs=2, space="PSUM"))
            lnsb = stg.enter_context(tc.tile_pool(name="lnsb", bufs=2))
            lnt_pool = stg.enter_context(tc.tile_pool(name="lnt4", bufs=4))
            rsb_pool = stg.enter_context(tc.tile_pool(name="rsb", bufs=2))
            drs_pool = stg.enter_context(
                tc.tile_pool(name="drs", bufs=2, space="DRAM"))

            def proj_ln(xT, w_sb_, out_sb, mc, w_ap, b_ap):
                lnts = {}
                for c in range(4):
                    pp = pp_pool.tile([P, 512], f32, tag="pp")
                    for k in range(8):
                        nc.tensor.matmul(
                            pp[:],
                            w_sb_[:, k, 128 * mc:128 * mc + 128],
                            xT[:, k, 512 * c:512 * c + 512],
                            start=(k == 0), stop=(k == 7))
                    xo = out_sb[:, mc, 512 * c:512 * c + 512]
                    nc.vector.tensor_copy(xo, pp[:])
                    sq = lnsb.tile([P, 512], bf16, tag="sq")
                    nc.vector.tensor_mul(sq[:], xo, xo)
                    vr = var_pool.tile([2, 512], f32, tag="vr")
                    nc.tensor.matmul(vr[:], red_sb[:], sq[:],
                                     start=True, stop=True)
                    lnt = lnt_pool.tile([2, 512], f32, tag="lnt",
                                        name=f"lnt{c}")
                    nc.scalar.activation(lnt[:], vr[:], AF.Ln, bias=eps_sb[:])
                    lnts[c] = lnt
                for c in range(4):
                    xo = out_sb[:, mc, 512 * c:512 * c + 512]
                    rs = lnsb.tile([2, 512], f32, tag="rs")
                    nc.scalar.activation(rs[:], lnts[c][:], AF.Exp,
                                         scale=-0.5)
                    rsd = drs_pool.tile([2, 512], f32)
                    nc.sync.dma_start(rsd[:], rs[:])
                    rsb = rsb_pool.tile([P, 512], f32)
                    nc.sync.dma_start(
                        rsb[0:64, :], rsd[0:1, :].to_broadcast((64, 512)))
                    nc.sync.dma_start(
                        rsb[64:128, :], rsd[1:2, :].to_broadcast((64, 512)))
                    nc.vector.scalar_tensor_tensor(
                        xo, xo, w_ap, rsb[:], op0=OP.mult, op1=OP.mult)
                    nc.vector.tensor_scalar_add(xo, xo, b_ap)

            proj_ln(ctxT_sb, wk_sb, kTn_sb, 0, knw_sb[:], knb_sb[:])
            proj_ln(embT_sb, wq_sb, qTn_sb, 0, qnw_sb[:], qnb_sb[:])
            proj_ln(ctxT_sb, wk_sb, kTn_sb, 1, knw_sb[:], knb_sb[:])
            proj_ln(embT_sb, wq_sb, qTn_sb, 1, qnw_sb[:], qnb_sb[:])

            def scores_unit(qh, hp):
                ats = {}
                for kt in range(16):
                    for h2 in range(2):
                        po = 64 * h2
                        sp = sp_pool.tile([P, 1024], f32, tag="sp")
                        for qn in range(2):
                            nc.tensor.matmul(
                                sp[:, 512 * qn:512 * qn + 512],
                                kTn_sb[po:po + 64, hp,
                                       128 * kt:128 * kt + 128],
                                qTn_sb[po:po + 64, hp,
                                       1024 * qh + 512 * qn:
                                       1024 * qh + 512 * qn + 512],
                                start=True, stop=True,
                                tile_position=(po, 0))
                        at = at_pool.tile([P, 1024], bf16)
                        nc.scalar.activation(at[:], sp[:], AF.Exp,
                                             scale=SCALE)
                        ats[(h2, kt)] = at
                return ats

            ats00 = scores_unit(0, 0)

            for sk in range(16):
                vp = pp_pool.tile([P, 256], f32, tag="pp")
                for k in range(8):
                    nc.tensor.matmul(
                        vp[:],
                        ctxT_sb[:, k, 128 * sk:128 * sk + 128],
                        wv_sb[:, k, :],
                        start=(k == 0), stop=(k == 7))
                nc.vector.tensor_copy(
                    v4[:, sk, :, 0:64],
                    vp[:].rearrange("p (g c) -> p g c", c=64))

        with ExitStack() as sb2:
            ot_pool = sb2.enter_context(
                tc.tile_pool(name="ot", bufs=4, space="PSUM"))
            den_pool = sb2.enter_context(tc.tile_pool(name="den", bufs=2))
            dend_pool = sb2.enter_context(
                tc.tile_pool(name="dend", bufs=2, space="DRAM"))
            obc_pool = sb2.enter_context(tc.tile_pool(name="obc", bufs=4))
            tmp_pool = sb2.enter_context(tc.tile_pool(name="tmp", bufs=2))
            y_pool = sb2.enter_context(tc.tile_pool(name="yo", bufs=3))

            def av_unit(qh, hp, ats):
                denall = den_pool.tile([65, 2048], f32, tag="denall")
                ots = {}
                for h2 in range(2):
                    for qc2 in range(2):
                        j = 2 * h2 + qc2
                        ot = ot_pool.tile([65, 512], f32, tag="ot",
                                          name=f"ot{j}")
                        h = 2 * hp + h2
                        for kt in range(16):
                            nc.tensor.matmul(
                                ot[:],
                                v_sb[:, kt, 65 * h:65 * h + 65],
                                ats[(h2, kt)][:, 512 * qc2:512 * qc2 + 512],
                                start=(kt == 0), stop=(kt == 15))
                        nc.vector.tensor_copy(
                            denall[64:65, 512 * j:512 * j + 512],
                            ot[64:65, :])
                        ots[j] = ot
                dend = dend_pool.tile([1, 2048], f32, tag="dend")
                nc.sync.dma_start(dend[:], denall[64:65, :])
                den4 = den_pool.tile([4, 512], f32, tag="den4")
                nc.sync.dma_start(
                    den4[:],
                    dend[0:1, :].rearrange("p (i c) -> (p i) c", c=512))
                den4r = den_pool.tile([4, 512], f32, tag="den4r")
                nc.vector.reciprocal_approx_fast(den4r[:], den4[:])
                dend2 = dend_pool.tile([4, 512], f32, tag="dend2")
                nc.sync.dma_start(dend2[:], den4r[:])
                for h2 in range(2):
                    for qc2 in range(2):
                        j = 2 * h2 + qc2
                        qc = 2 * qh + qc2
                        obc = obc_pool.tile([64, 512], f32)
                        nc.sync.dma_start(
                            obc[:], dend2[j:j + 1, :].to_broadcast((64, 512)))
                        if h2 == 0:
                            nc.vector.tensor_mul(
                                oT2_sb[0:64, hp, 512 * qc:512 * qc + 512],
                                ots[j][0:64, :], obc[:])
                        else:
                            tmp = tmp_pool.tile([64, 512], bf16)
                            nc.vector.tensor_mul(tmp[:], ots[j][0:64, :],
                                                 obc[:])
                            nc.sync.dma_start(
                                oT2_sb[64:128, hp, 512 * qc:512 * qc + 512],
                                tmp[:])

            def out_proj(qh):
                for qt in range(8):
                    q0 = 1024 * qh + 128 * qt
                    yp = sp_pool.tile([P, 1024], f32, tag="sp")
                    for hp in range(2):
                        for e2 in range(2):
                            nc.tensor.matmul(
                                yp[:, 512 * e2:512 * e2 + 512],
                                oT2_sb[:, hp, q0:q0 + 128],
                                wu2_sb[:, hp, 512 * e2:512 * e2 + 512],
                                start=(hp == 0), stop=(hp == 1))
                    ysb = y_pool.tile([P, 1024], bf16)
                    nc.vector.tensor_copy(ysb[:], yp[:])
                    nc.sync.dma_start(y_d[q0:q0 + 128, :], ysb[:])

            av_unit(0, 0, ats00)
            ats01 = scores_unit(0, 1)
            av_unit(0, 1, ats01)
            ats10 = scores_unit(1, 0)
            av_unit(1, 0, ats10)
            out_proj(0)
            ats11 = scores_unit(1, 1)
            av_unit(1, 1, ats11)
            out_proj(1)

    nc.compile()
    return nc


def _host_inputs(emb, context, Wq, Wk, Wv, Wu, qn_w, qn_b, kn_w, kn_b):
    bf16 = ml_dtypes.bfloat16
    redblk = np.zeros((P, 2), np.float32)
    redblk[0:64, 0] = 1.0 / 64.0
    redblk[64:128, 1] = 1.0 / 64.0
    redblk = redblk.astype(bf16)

    def center(Wrows):
        Wh = Wrows.reshape(HG, D, Wrows.shape[1])
        return (Wh - Wh.mean(axis=1, keepdims=True)).reshape(Wrows.shape)

    tile2 = lambda w: np.ascontiguousarray(
        np.tile(np.asarray(w, np.float32), 2)[:, None])

    in_maps = []
    for c in range(8):
        b, hg = divmod(c, 4)
        rows = slice(INNER_C * hg, INNER_C * (hg + 1))
        wu2 = np.stack([
            np.ascontiguousarray(Wu[:, INNER_C * hg + 128 * hp:
                                    INNER_C * hg + 128 * hp + 128].T)
            for hp in range(2)], axis=1)
        in_maps.append({
            "embT": np.ascontiguousarray(emb[b].T).astype(bf16),
            "ctxT": np.ascontiguousarray(context[b].T).astype(bf16),
            "wqT": np.ascontiguousarray(center(Wq[rows]).T).astype(bf16),
            "wkT": np.ascontiguousarray(center(Wk[rows]).T).astype(bf16),
            "wvT": np.ascontiguousarray(Wv[rows].T).astype(bf16),
            "wu2": np.ascontiguousarray(wu2).astype(bf16),
            "redblk": redblk,
            "qnw": tile2(qn_w),
            "qnb": tile2(qn_b),
            "knw": tile2(kn_w),
            "knb": tile2(kn_b),
        })
    return in_maps


def kernel(emb, context, Wq, Wk, Wv, Wu, bu, qn_w, qn_b, kn_w, kn_b):
    from concourse.bass_utils import run_bass_kernel_spmd

    global _cached_nc
    if _cached_nc is None:
        _cached_nc = _build()
    nc = _cached_nc

    in_maps = _host_inputs(np.asarray(emb, np.float32),
                           np.asarray(context, np.float32),
                           np.asarray(Wq), np.asarray(Wk), np.asarray(Wv),
                           np.asarray(Wu), np.asarray(qn_w), np.asarray(qn_b),
                           np.asarray(kn_w), np.asarray(kn_b))

    trace = bool(os.environ.get("KERNEL_TRACE"))
    res = run_bass_kernel_spmd(nc, in_maps, core_ids=list(range(8)),
                               trace=trace)
    if trace:
        print(f"HW exec time: {res.exec_time_ns} ns")

    out = np.zeros((B, SQ, EMB), np.float32)
    for c in range(8):
        out[c // 4] += np.asarray(res.results[c]["ypart"], np.float32)
    out += np.asarray(bu, np.float32)[None, None, :]
    return out


if __name__ == "__main__":
    pass
